# revision 12
# baseline (speedup 1.0000x reference)
"""Capsule-routing kernel (einsum bni,nkdi,nk->bkd + squash) on 8 trn2 cores.

Sharding: over the contraction axis n (2048 -> 256 per core).  Each core
reads only its slice of x and W -- every input byte is read exactly once
machine-wide.  Each core emits a partial s[b,(k,d)] over its n-slice; the
host sums the 8 partials and applies the tiny squash nonlinearity.

V4 pipeline (vs the 40.5us baseline / 35.4us bf16-streaming version):
  - W is uploaded as INT8 with per-(n,k) host-computed scales (softmax(R)
    folded into the scale), halving the dominant DMA stream.  A DVE
    tensor_mul per chunk dequantizes int8 -> bf16 while applying the
    scale; measured Frobenius error ~7e-3, well under the 2e-2 gate.
  - ALL input DMAs ride ONE HWDGE queue (nc.sync) issued in exactly the
    matmul consumption order.  Two-queue splits round-robin at packet
    granularity and destroy the arrival order.
  - DMA-completion "touchers" live on the ACT (scalar) engine so the DVE
    is free for dequant; each instruction carries at most one sem wait
    (walrus build limit).
  - a burst of dummy matmuls on a memset tile trips the PE HAM clock
    gate (4096-cycle activity window) so real matmuls run at 2.4 GHz.
  - matmul order (t, i, h); the last W chunk runs h0-first so acc0
    closes early and its output pipeline overlaps the remaining MMs.
  - output: bf16 partials; acc0 via DVE copy + SWDGE DMA, acc1 via ACT
    copy + ACT-issued HWDGE DMA (copy and DMA on the same engine means
    program order replaces a second sem wait).
"""

import os
import sys

import numpy as np

if "/opt/trn_rl_repo" not in sys.path:
    sys.path.insert(0, "/opt/trn_rl_repo")

import bass_rust as _bass_rust
import concourse.bass as bass
import concourse.mybir as mybir
import ml_dtypes
from concourse.bass_utils import run_bass_kernel_spmd
from concourse.tile import TileContext

NCORES = 8
B, N, I = 256, 2048, 16
K, D = 32, 16
NL = N // NCORES  # 256 n-values per core
KD = K * D  # 512
F_W = I * K * D  # 8192   (i-major W layout)
F_X = I * B  # 4096      (x^T layout: [n, i, B])
EPS = 1e-7

FP32 = mybir.dt.float32
BF16 = mybir.dt.bfloat16
INT8 = mybir.dt.int8
NPBF16 = ml_dtypes.bfloat16

# Split Tile's multi-wait kernel-tail drain into a chain of single-wait
# drains (program order on the sync sequencer makes the chain equivalent).
if not getattr(TileContext, "_split_drain_patched", False):

    def _split_drain_and_barrier(self, tick_clock, wait_clock):
        gc = tick_clock.global_clock
        vals = list(gc)
        for j, v in enumerate(vals):
            if v > 0:
                sub = [0] * len(vals)
                sub[j] = v
                d = self.nc.sync.drain()
                wait_clock.add_sem_waits(
                    d.ins,
                    _bass_rust.ScopedClock({None: _bass_rust.VectorClock(sub)}),
                )
        self.nc.all_engine_barrier()
        assert self.sems is not None
        popped = self.nc._tile_sem_poison_stack.pop()
        assert popped is self._sem_poison
        self.nc.clear_and_free_semaphores(list(self.sems.allocated().values()))

    TileContext._drain_and_barrier = _split_drain_and_barrier
    TileContext._split_drain_patched = True

# x pieces / W chunks per n-half, in units of i (boundaries align with the
# matmul i-loop).  Early chunks small so the PE starts early, last chunk
# small so the post-DMA trail is short.
X_PIECES = [[(0, 6), (6, 16)], [(0, 6), (6, 16)]]
W_PIECES = [[(0, 2), (2, 6), (6, 10), (10, 16)], [(0, 2), (2, 9), (9, 13), (13, 16)]]
N_WARM_MM = 10  # dummy matmuls to trip the HAM clock gate


def build_bass() -> bass.Bass:
    nc = bass.Bass()
    x_d = nc.dram_tensor("xs", [NL, F_X], BF16, kind="ExternalInput")
    w_d = nc.dram_tensor("ws", [NL, F_W], INT8, kind="ExternalInput")
    s_d = nc.dram_tensor("sc", [NL, KD], BF16, kind="ExternalInput")
    o_d = nc.dram_tensor("out", [B, KD], BF16, kind="ExternalOutput")

    with TileContext(nc) as tc:
        with (
            tc.tile_pool(name="big", bufs=1) as big,
            tc.tile_pool(name="ps_warm", bufs=1, space="PSUM") as ps_warm,
            tc.tile_pool(name="ps_acc", bufs=1, space="PSUM") as ps_acc,
        ):
            # garbage tile for the PE warm-up burst
            gtile = big.tile([128, 512], BF16, tag="gt")
            nc.gpsimd.memset(gtile[:], 0.0)

            xb = [big.tile([128, F_X], BF16, tag=f"x{t}", name=f"x{t}") for t in range(2)]
            ws = [big.tile([128, F_W], INT8, tag=f"ws{t}", name=f"ws{t}") for t in range(2)]
            wb = [big.tile([128, F_W], BF16, tag=f"wb{t}", name=f"wb{t}") for t in range(2)]
            sc = big.tile([128, 2 * KD], BF16, tag="sc")

            # ---- input DMAs: one HWDGE queue, issued in consumption order
            # (scales first, then per n-half x pieces and W chunks merged by
            # first-use i; x before W at equal i). ----
            sched = [("s", 0, 0, 0)]
            for t in range(2):
                ev = [("x", t, lo, hi) for (lo, hi) in X_PIECES[t]]
                ev += [("w", t, lo, hi) for (lo, hi) in W_PIECES[t]]
                ev.sort(key=lambda e: (e[2], 0 if e[0] == "x" else 1))
                sched += ev
            for kind, t, lo, hi in sched:
                if kind == "s":
                    nc.sync.dma_start(
                        out=sc[:], in_=s_d.rearrange("(t p) f -> p t f", t=2)
                    )
                elif kind == "x":
                    nc.sync.dma_start(
                        out=xb[t][:, lo * B : hi * B],
                        in_=x_d[t * 128 : (t + 1) * 128, lo * B : hi * B],
                    )
                else:
                    nc.sync.dma_start(
                        out=ws[t][:, lo * KD : hi * KD],
                        in_=w_d[t * 128 : (t + 1) * 128, lo * KD : hi * KD],
                    )

            # ---- one DVE toucher absorbs the scale DMA into DVE program
            # order, so each dequant carries only its own W-chunk DMA wait
            # (walrus allows one wait per instruction).  x/W waits on the
            # matmuls split naturally across the LDW/MM pair. ----
            with tc.high_priority():
                tt = big.tile([128, 1], BF16, tag="t_sc")
                nc.vector.tensor_copy(tt[:], sc[:, 0:1])

            # ---- PE warm-up: dummy matmuls on the memset tile keep the
            # HAM activity window busy so real matmuls start at 2.4 GHz ----
            warm_ps = ps_warm.tile([128, 512], FP32, tag="warmps")
            for _ in range(N_WARM_MM):
                nc.tensor.matmul(
                    warm_ps[:], gtile[:, 0:128], gtile[:], start=True, stop=True
                )

            # ---- dequant: per W chunk, wb = bf16(ws_int8 * scale), scale
            # broadcast over i via a step-0 AP (softmax(R) and the int8
            # scale are folded host-side) ----
            for t in range(2):
                for lo, hi in W_PIECES[t]:
                    sl_in = ws[t][:, lo * KD : hi * KD].rearrange(
                        "p (i f) -> p i f", f=KD
                    )
                    sl_out = wb[t][:, lo * KD : hi * KD].rearrange(
                        "p (i f) -> p i f", f=KD
                    )
                    r_sl = sc[:, t * KD : (t + 1) * KD]
                    r_b = bass.AP(
                        tensor=r_sl.tensor,
                        offset=r_sl.offset,
                        ap=[r_sl.ap[0], [0, hi - lo], [1, KD]],
                    )
                    nc.vector.tensor_mul(sl_out, sl_in, r_b)

            # ---- main matmuls: (t, i, h); acc_h[b, (k d)] accumulates all
            # 32 contraction steps of B-half h.  The last W chunk's i's run
            # h0-first so acc0 closes early. ----
            last_lo = W_PIECES[1][-1][0]
            order = []
            for t in range(2):
                for i in range(I):
                    if t == 1 and i == last_lo:
                        order += [(1, j, 0) for j in range(last_lo, I)]
                        order += [(1, j, 1) for j in range(last_lo, I)]
                        break
                    order += [(t, i, 0), (t, i, 1)]
            first_of = {0: None, 1: None}
            last_of = {0: None, 1: None}
            for idx, (t, i, h) in enumerate(order):
                if first_of[h] is None:
                    first_of[h] = idx
                last_of[h] = idx

            accs = [
                ps_acc.tile([128, KD], FP32, tag=f"acc{h}", name=f"acc{h}")
                for h in range(2)
            ]
            for idx, (t, i, h) in enumerate(order):
                rhs = wb[t][:, i * KD : (i + 1) * KD]
                lhsT = xb[t][:, i * B + h * 128 : i * B + (h + 1) * 128]
                nc.tensor.matmul(
                    accs[h][:],
                    lhsT,
                    rhs,
                    start=(idx == first_of[h]),
                    stop=(idx == last_of[h]),
                )

            # ---- output: bf16 partials; PSUM->SBUF copies in parallel on
            # DVE and ACT, one SWDGE DMA per B-half (fresh DMASW lanes;
            # each DMA carries exactly one wait -- its own copy) ----
            o_sb = big.tile([128, 2 * KD], BF16, tag="osb")
            nc.vector.tensor_copy(o_sb[:, 0:KD], accs[0][:])
            nc.gpsimd.dma_start(out=o_d[0:128, :], in_=o_sb[:, 0:KD])
            nc.scalar.copy(o_sb[:, KD : 2 * KD], accs[1][:])
            nc.gpsimd.dma_start(out=o_d[128:256, :], in_=o_sb[:, KD : 2 * KD])

    return nc


_CACHE: dict = {}

# test.py sets these for profiling; harness never touches them.
LAST_RESULTS = None


def _trace_kwargs():
    if os.environ.get("BASS_KERNEL_TRACE") == "1":
        cores = os.environ.get("BASS_KERNEL_TRACE_CORES", "0")
        return dict(trace=True, trace_cores=[int(c) for c in cores.split(",")])
    return {}


def kernel(x: np.ndarray, W: np.ndarray, R: np.ndarray) -> np.ndarray:
    global LAST_RESULTS
    x = np.asarray(x, dtype=np.float32)
    W = np.asarray(W, dtype=np.float32)
    R = np.asarray(R, dtype=np.float32)

    # softmax over n (65K elements -- host)
    Rm = R.max(axis=0, keepdims=True)
    e = np.exp(R - Rm)
    Rs = (e / e.sum(axis=0, keepdims=True)).astype(np.float32)

    # upload layouts: x^T as [n, i, B] bf16; W quantized to int8 with a
    # per-(n,k) scale, i-major as [n, i, k, d]; the dequant scale
    # (amax/127 * softmax(R)) pre-broadcast over d as [n, (k d)] bf16
    Xp = np.ascontiguousarray(x.transpose(1, 2, 0)).reshape(N, F_X).astype(NPBF16)
    amax = np.abs(W).max(axis=(2, 3), keepdims=True)  # [n, k, 1, 1]
    amax = np.maximum(amax, 1e-20)
    Wq = np.clip(np.round(W / amax * 127.0), -127, 127).astype(np.int8)
    Wp = np.ascontiguousarray(Wq.transpose(0, 3, 1, 2)).reshape(N, F_W)
    scale = (amax[:, :, 0, 0] / 127.0) * Rs  # [n, k]
    Sp = np.ascontiguousarray(np.repeat(scale, D, axis=1)).astype(NPBF16)
    in_maps = []
    for c in range(NCORES):
        sl = slice(c * NL, (c + 1) * NL)
        in_maps.append({"xs": Xp[sl], "ws": Wp[sl], "sc": Sp[sl]})

    if "nc" not in _CACHE:
        _CACHE["nc"] = build_bass()
    nc = _CACHE["nc"]

    res = run_bass_kernel_spmd(
        nc, in_maps, core_ids=list(range(NCORES)), **_trace_kwargs()
    )
    LAST_RESULTS = res

    s = np.zeros((B, KD), np.float32)
    for r in res.results:
        s += r["out"].astype(np.float32)
    s = s.reshape(B, K, D)
    sq = np.sum(np.square(s), axis=-1, keepdims=True) + EPS
    v = (np.sqrt(sq) / (1.0 + sq)) * s
    return v.astype(np.float32)


if __name__ == "__main__":
    rng = np.random.default_rng(0)
    x = rng.standard_normal((B, N, I), dtype=np.float32)
    W = (rng.standard_normal((N, K, D, I), dtype=np.float32) * 0.05).astype(np.float32)
    R = rng.standard_normal((N, K), dtype=np.float32)
    out = kernel(x, W, R)
    print("out", out.shape, out.dtype, float(np.abs(out).mean()))


# revision 13
# speedup vs baseline: 1.0986x; 1.0986x over previous
"""Capsule-routing kernel (einsum bni,nkdi,nk->bkd + squash) on 8 trn2 cores.

Sharding: over the contraction axis n (2048 -> 256 per core).  Each core
reads only its slice of x and W -- every input byte is read exactly once
machine-wide.  Each core emits a partial s[b,(k,d)] over its n-slice; the
host sums the 8 partials and applies the tiny squash nonlinearity.

Pipeline (evolved from a 40.5us baseline via trace analysis):
  - softmax(R) is folded into W on the HOST during shard marshalling, so
    the device kernel has no scaling stage at all -- matmuls consume W
    straight from the DMA tiles.  (An int8-W variant was measured: the
    DVE dequant runs without perf modes on 1-byte operands and becomes a
    30us bottleneck, so bf16 W wins despite 2x the bytes.)
  - ALL input DMAs ride ONE HWDGE queue (nc.sync) issued in exactly the
    matmul consumption order; concurrent queues round-robin per packet
    and destroy arrival order.  11 DMAs keeps sem-lane reuse (8 lanes)
    from stalling the sync sequencer: a reused lane puts the
    predecessor-completion wait on the DMA issue op itself.
  - x0's first piece is tiny so the PE's first LDW wait (DMA receipt is
    ~2us after last byte) clears early.
  - a burst of dummy matmuls on a memset tile trips the PE HAM clock
    gate so real matmuls run at 2.4 GHz from the start.
  - matmul order (t, i, h) with one PSUM accumulator per B-half; the
    last W chunk runs h0-first so acc0 closes early and its output
    pipeline overlaps the remaining h1 matmuls.
  - output: bf16 partials; PSUM->SBUF copies in parallel on DVE and ACT,
    one SWDGE DMA per B-half (each instruction carries at most the ONE
    sem wait this walrus build allows; LDW/MM naturally split the x/W
    DMA waits).

Precision: bf16 matmul with fp32 PSUM accumulation; bf16 partial sums.
Measured Frobenius rel err ~3e-3 (gate 2e-2).
"""

import os
import sys

import numpy as np

if "/opt/trn_rl_repo" not in sys.path:
    sys.path.insert(0, "/opt/trn_rl_repo")

import bass_rust as _bass_rust
import concourse.bass as bass
import concourse.mybir as mybir
import ml_dtypes
from concourse.bass_utils import run_bass_kernel_spmd
from concourse.tile import TileContext

NCORES = 8
B, N, I = 256, 2048, 16
K, D = 32, 16
NL = N // NCORES  # 256 n-values per core
KD = K * D  # 512
F_W = I * K * D  # 8192   (i-major W layout)
F_X = I * B  # 4096      (x^T layout: [n, i, B])
EPS = 1e-7

FP32 = mybir.dt.float32
BF16 = mybir.dt.bfloat16
NPBF16 = ml_dtypes.bfloat16

# Split Tile's multi-wait kernel-tail drain into a chain of single-wait
# drains (program order on the sync sequencer makes the chain equivalent).
if not getattr(TileContext, "_split_drain_patched", False):

    def _split_drain_and_barrier(self, tick_clock, wait_clock):
        gc = tick_clock.global_clock
        vals = list(gc)
        for j, v in enumerate(vals):
            if v > 0:
                sub = [0] * len(vals)
                sub[j] = v
                d = self.nc.sync.drain()
                wait_clock.add_sem_waits(
                    d.ins,
                    _bass_rust.ScopedClock({None: _bass_rust.VectorClock(sub)}),
                )
        self.nc.all_engine_barrier()
        assert self.sems is not None
        popped = self.nc._tile_sem_poison_stack.pop()
        assert popped is self._sem_poison
        self.nc.clear_and_free_semaphores(list(self.sems.allocated().values()))

    TileContext._drain_and_barrier = _split_drain_and_barrier
    TileContext._split_drain_patched = True

# x pieces / W chunks per n-half, in units of i (boundaries align with the
# matmul i-loop).  First pieces small so the PE starts early; last chunk
# small so the post-DMA trail is short; 11 total to limit lane reuse.
X_PIECES = [[(0, 2), (2, 16)], [(0, 6), (6, 16)]]
W_PIECES = [[(0, 2), (2, 6), (6, 16)], [(0, 2), (2, 7), (7, 12), (12, 16)]]
N_WARM_MM = 10  # dummy matmuls to trip the HAM clock gate


def build_bass() -> bass.Bass:
    nc = bass.Bass()
    x_d = nc.dram_tensor("xs", [NL, F_X], BF16, kind="ExternalInput")
    w_d = nc.dram_tensor("ws", [NL, F_W], BF16, kind="ExternalInput")
    o_d = nc.dram_tensor("out", [B, KD], BF16, kind="ExternalOutput")

    with TileContext(nc) as tc:
        with (
            tc.tile_pool(name="big", bufs=1) as big,
            tc.tile_pool(name="ps_warm", bufs=1, space="PSUM") as ps_warm,
            tc.tile_pool(name="ps_acc", bufs=1, space="PSUM") as ps_acc,
        ):
            # garbage tile for the PE warm-up burst
            gtile = big.tile([128, 512], BF16, tag="gt")
            nc.gpsimd.memset(gtile[:], 0.0)

            xb = [big.tile([128, F_X], BF16, tag=f"x{t}", name=f"x{t}") for t in range(2)]
            wb = [big.tile([128, F_W], BF16, tag=f"w{t}", name=f"w{t}") for t in range(2)]

            # ---- input DMAs: one HWDGE queue, issued in consumption order
            # (per n-half, x pieces and W chunks merged by first-use i; x
            # before W at equal i since the LDW precedes the MM) ----
            sched = []
            for t in range(2):
                ev = [("x", t, lo, hi) for (lo, hi) in X_PIECES[t]]
                ev += [("w", t, lo, hi) for (lo, hi) in W_PIECES[t]]
                ev.sort(key=lambda e: (e[2], 0 if e[0] == "x" else 1))
                sched += ev
            for kind, t, lo, hi in sched:
                if kind == "x":
                    nc.sync.dma_start(
                        out=xb[t][:, lo * B : hi * B],
                        in_=x_d[t * 128 : (t + 1) * 128, lo * B : hi * B],
                    )
                else:
                    nc.sync.dma_start(
                        out=wb[t][:, lo * KD : hi * KD],
                        in_=w_d[t * 128 : (t + 1) * 128, lo * KD : hi * KD],
                    )

            # ---- PE warm-up: dummy matmuls on the memset tile keep the
            # HAM activity window busy so real matmuls start at 2.4 GHz ----
            warm_ps = ps_warm.tile([128, 512], FP32, tag="warmps")
            for _ in range(N_WARM_MM):
                nc.tensor.matmul(
                    warm_ps[:], gtile[:, 0:128], gtile[:], start=True, stop=True
                )

            # ---- main matmuls: (t, i, h); acc_h[b, (k d)] accumulates all
            # 32 contraction steps of B-half h.  The last W chunk's i's run
            # h0-first so acc0 closes early. ----
            last_lo = W_PIECES[1][-1][0]
            order = []
            for t in range(2):
                for i in range(I):
                    if t == 1 and i == last_lo:
                        order += [(1, j, 0) for j in range(last_lo, I)]
                        order += [(1, j, 1) for j in range(last_lo, I)]
                        break
                    order += [(t, i, 0), (t, i, 1)]
            first_of = {0: None, 1: None}
            last_of = {0: None, 1: None}
            for idx, (t, i, h) in enumerate(order):
                if first_of[h] is None:
                    first_of[h] = idx
                last_of[h] = idx

            accs = [
                ps_acc.tile([128, KD], FP32, tag=f"acc{h}", name=f"acc{h}")
                for h in range(2)
            ]
            for idx, (t, i, h) in enumerate(order):
                rhs = wb[t][:, i * KD : (i + 1) * KD]
                lhsT = xb[t][:, i * B + h * 128 : i * B + (h + 1) * 128]
                nc.tensor.matmul(
                    accs[h][:],
                    lhsT,
                    rhs,
                    start=(idx == first_of[h]),
                    stop=(idx == last_of[h]),
                )

            # ---- output: bf16 partials; PSUM->SBUF copies in parallel on
            # DVE and ACT, one SWDGE DMA per B-half (fresh DMASW lanes;
            # each DMA carries exactly one wait -- its own copy) ----
            o_sb = big.tile([128, 2 * KD], BF16, tag="osb")
            nc.vector.tensor_copy(o_sb[:, 0:KD], accs[0][:])
            nc.gpsimd.dma_start(out=o_d[0:128, :], in_=o_sb[:, 0:KD])
            nc.scalar.copy(o_sb[:, KD : 2 * KD], accs[1][:])
            nc.gpsimd.dma_start(out=o_d[128:256, :], in_=o_sb[:, KD : 2 * KD])

    return nc


_CACHE: dict = {}

# test.py sets these for profiling; harness never touches them.
LAST_RESULTS = None


def _trace_kwargs():
    if os.environ.get("BASS_KERNEL_TRACE") == "1":
        cores = os.environ.get("BASS_KERNEL_TRACE_CORES", "0")
        return dict(trace=True, trace_cores=[int(c) for c in cores.split(",")])
    return {}


def kernel(x: np.ndarray, W: np.ndarray, R: np.ndarray) -> np.ndarray:
    global LAST_RESULTS
    x = np.asarray(x, dtype=np.float32)
    W = np.asarray(W, dtype=np.float32)
    R = np.asarray(R, dtype=np.float32)

    # softmax over n (65K elements -- host)
    Rm = R.max(axis=0, keepdims=True)
    e = np.exp(R - Rm)
    Rs = (e / e.sum(axis=0, keepdims=True)).astype(np.float32)

    # upload layouts: x^T as [n, i, B]; W scaled by softmax(R) and laid out
    # i-major as [n, i, k, d]; both in the kernel's bf16 compute precision
    Xp = np.ascontiguousarray(x.transpose(1, 2, 0)).reshape(N, F_X).astype(NPBF16)
    Wsc = W * Rs[:, :, None, None]
    Wp = np.ascontiguousarray(Wsc.transpose(0, 3, 1, 2)).reshape(N, F_W).astype(NPBF16)
    in_maps = []
    for c in range(NCORES):
        sl = slice(c * NL, (c + 1) * NL)
        in_maps.append({"xs": Xp[sl], "ws": Wp[sl]})

    if "nc" not in _CACHE:
        _CACHE["nc"] = build_bass()
    nc = _CACHE["nc"]

    res = run_bass_kernel_spmd(
        nc, in_maps, core_ids=list(range(NCORES)), **_trace_kwargs()
    )
    LAST_RESULTS = res

    s = np.zeros((B, KD), np.float32)
    for r in res.results:
        s += r["out"].astype(np.float32)
    s = s.reshape(B, K, D)
    sq = np.sum(np.square(s), axis=-1, keepdims=True) + EPS
    v = (np.sqrt(sq) / (1.0 + sq)) * s
    return v.astype(np.float32)


if __name__ == "__main__":
    rng = np.random.default_rng(0)
    x = rng.standard_normal((B, N, I), dtype=np.float32)
    W = (rng.standard_normal((N, K, D, I), dtype=np.float32) * 0.05).astype(np.float32)
    R = rng.standard_normal((N, K), dtype=np.float32)
    out = kernel(x, W, R)
    print("out", out.shape, out.dtype, float(np.abs(out).mean()))


# revision 14
# speedup vs baseline: 1.1063x; 1.0071x over previous
"""Capsule-routing kernel (einsum bni,nkdi,nk->bkd + squash) on 8 trn2 cores.

Sharding: over the contraction axis n (2048 -> 256 per core).  Each core
reads only its slice of x and W -- every input byte is read exactly once
machine-wide.  Each core emits a partial s[b,(k,d)] over its n-slice; the
host sums the 8 partials and applies the tiny squash nonlinearity.

Pipeline (evolved from a 40.5us baseline via trace analysis):
  - softmax(R) is folded into W on the HOST during shard marshalling, so
    the device kernel has no scaling stage at all -- matmuls consume W
    straight from the DMA tiles.  (An int8-W variant was measured: the
    DVE dequant runs without perf modes on 1-byte operands and becomes a
    30us bottleneck, so bf16 W wins despite 2x the bytes.)
  - ALL input DMAs ride ONE HWDGE queue (nc.sync) issued in exactly the
    matmul consumption order; concurrent queues round-robin per packet
    and destroy arrival order.  11 DMAs keeps sem-lane reuse (8 lanes)
    from stalling the sync sequencer: a reused lane puts the
    predecessor-completion wait on the DMA issue op itself.
  - x0's first piece is tiny so the PE's first LDW wait (DMA receipt is
    ~2us after last byte) clears early.
  - a burst of dummy matmuls on a memset tile trips the PE HAM clock
    gate so real matmuls run at 2.4 GHz from the start.
  - matmul order (t, i, h) with one PSUM accumulator per B-half; the
    last W chunk runs h0-first so acc0 closes early and its output
    pipeline overlaps the remaining h1 matmuls.
  - output: bf16 partials; PSUM->SBUF copies in parallel on DVE and ACT,
    one SWDGE DMA per B-half (each instruction carries at most the ONE
    sem wait this walrus build allows; LDW/MM naturally split the x/W
    DMA waits).

Precision: bf16 matmul with fp32 PSUM accumulation; bf16 partial sums.
Measured Frobenius rel err ~3e-3 (gate 2e-2).
"""

import os
import sys

import numpy as np

if "/opt/trn_rl_repo" not in sys.path:
    sys.path.insert(0, "/opt/trn_rl_repo")

import bass_rust as _bass_rust
import concourse.bass as bass
import concourse.mybir as mybir
import ml_dtypes
from concourse.bass_utils import run_bass_kernel_spmd
from concourse.tile import TileContext

NCORES = 8
B, N, I = 256, 2048, 16
K, D = 32, 16
NL = N // NCORES  # 256 n-values per core
KD = K * D  # 512
F_W = I * K * D  # 8192   (i-major W layout)
F_X = I * B  # 4096      (x^T layout: [n, i, B])
EPS = 1e-7

FP32 = mybir.dt.float32
BF16 = mybir.dt.bfloat16
NPBF16 = ml_dtypes.bfloat16

# Split Tile's multi-wait kernel-tail drain into a chain of single-wait
# drains (program order on the sync sequencer makes the chain equivalent).
if not getattr(TileContext, "_split_drain_patched", False):

    def _split_drain_and_barrier(self, tick_clock, wait_clock):
        gc = tick_clock.global_clock
        vals = list(gc)
        for j, v in enumerate(vals):
            if v > 0:
                sub = [0] * len(vals)
                sub[j] = v
                d = self.nc.sync.drain()
                wait_clock.add_sem_waits(
                    d.ins,
                    _bass_rust.ScopedClock({None: _bass_rust.VectorClock(sub)}),
                )
        self.nc.all_engine_barrier()
        assert self.sems is not None
        popped = self.nc._tile_sem_poison_stack.pop()
        assert popped is self._sem_poison
        self.nc.clear_and_free_semaphores(list(self.sems.allocated().values()))

    TileContext._drain_and_barrier = _split_drain_and_barrier
    TileContext._split_drain_patched = True

# x pieces / W chunks per n-half, in units of i (boundaries align with the
# matmul i-loop).  x0 whole and first: it keeps the ring busy while later
# issues queue, and W then streams gap-free behind it (x pieces placed
# between W chunks delay them and stall the PE into a HAM re-throttle).
# Early W chunks small so the PE starts early; last chunk 2 i's so the
# post-stream trail (receipt + remaining matmuls) is short.
X_PIECES = [[(0, 16)], [(0, 6), (6, 16)]]
W_PIECES = [[(0, 2), (2, 6), (6, 10), (10, 16)], [(0, 2), (2, 6), (6, 10), (10, 14), (14, 16)]]
N_WARM_MM = 10  # dummy matmuls to trip the HAM clock gate


def build_bass() -> bass.Bass:
    nc = bass.Bass()
    x_d = nc.dram_tensor("xs", [NL, F_X], BF16, kind="ExternalInput")
    w_d = nc.dram_tensor("ws", [NL, F_W], BF16, kind="ExternalInput")
    o_d = nc.dram_tensor("out", [B, KD], BF16, kind="ExternalOutput")

    with TileContext(nc) as tc:
        with (
            tc.tile_pool(name="big", bufs=1) as big,
            tc.tile_pool(name="ps_warm", bufs=1, space="PSUM") as ps_warm,
            tc.tile_pool(name="ps_acc", bufs=1, space="PSUM") as ps_acc,
        ):
            # garbage tile for the PE warm-up burst
            gtile = big.tile([128, 512], BF16, tag="gt")
            nc.gpsimd.memset(gtile[:], 0.0)

            xb = [big.tile([128, F_X], BF16, tag=f"x{t}", name=f"x{t}") for t in range(2)]
            wb = [big.tile([128, F_W], BF16, tag=f"w{t}", name=f"w{t}") for t in range(2)]

            # ---- input DMAs: one HWDGE queue, issued in consumption order
            # (per n-half, x pieces and W chunks merged by first-use i; x
            # before W at equal i since the LDW precedes the MM) ----
            sched = []
            for t in range(2):
                ev = [("x", t, lo, hi) for (lo, hi) in X_PIECES[t]]
                ev += [("w", t, lo, hi) for (lo, hi) in W_PIECES[t]]
                ev.sort(key=lambda e: (e[2], 0 if e[0] == "x" else 1))
                sched += ev
            for kind, t, lo, hi in sched:
                if kind == "x":
                    nc.sync.dma_start(
                        out=xb[t][:, lo * B : hi * B],
                        in_=x_d[t * 128 : (t + 1) * 128, lo * B : hi * B],
                    )
                else:
                    nc.sync.dma_start(
                        out=wb[t][:, lo * KD : hi * KD],
                        in_=w_d[t * 128 : (t + 1) * 128, lo * KD : hi * KD],
                    )

            # ---- PE warm-up: dummy matmuls on the memset tile keep the
            # HAM activity window busy so real matmuls start at 2.4 GHz ----
            warm_ps = ps_warm.tile([128, 512], FP32, tag="warmps")
            for _ in range(N_WARM_MM):
                nc.tensor.matmul(
                    warm_ps[:], gtile[:, 0:128], gtile[:], start=True, stop=True
                )

            # ---- main matmuls: (t, i, h); acc_h[b, (k d)] accumulates all
            # 32 contraction steps of B-half h.  The last W chunk's i's run
            # h0-first so acc0 closes early. ----
            last_lo = W_PIECES[1][-1][0]
            order = []
            for t in range(2):
                for i in range(I):
                    if t == 1 and i == last_lo:
                        order += [(1, j, 0) for j in range(last_lo, I)]
                        order += [(1, j, 1) for j in range(last_lo, I)]
                        break
                    order += [(t, i, 0), (t, i, 1)]
            first_of = {0: None, 1: None}
            last_of = {0: None, 1: None}
            for idx, (t, i, h) in enumerate(order):
                if first_of[h] is None:
                    first_of[h] = idx
                last_of[h] = idx

            accs = [
                ps_acc.tile([128, KD], FP32, tag=f"acc{h}", name=f"acc{h}")
                for h in range(2)
            ]
            for idx, (t, i, h) in enumerate(order):
                rhs = wb[t][:, i * KD : (i + 1) * KD]
                lhsT = xb[t][:, i * B + h * 128 : i * B + (h + 1) * 128]
                nc.tensor.matmul(
                    accs[h][:],
                    lhsT,
                    rhs,
                    start=(idx == first_of[h]),
                    stop=(idx == last_of[h]),
                )

            # ---- output: bf16 partials; PSUM->SBUF copies in parallel on
            # DVE and ACT, one SWDGE DMA per B-half (fresh DMASW lanes;
            # each DMA carries exactly one wait -- its own copy) ----
            o_sb = big.tile([128, 2 * KD], BF16, tag="osb")
            nc.vector.tensor_copy(o_sb[:, 0:KD], accs[0][:])
            nc.gpsimd.dma_start(out=o_d[0:128, :], in_=o_sb[:, 0:KD])
            nc.scalar.copy(o_sb[:, KD : 2 * KD], accs[1][:])
            nc.gpsimd.dma_start(out=o_d[128:256, :], in_=o_sb[:, KD : 2 * KD])

    return nc


_CACHE: dict = {}

# test.py sets these for profiling; harness never touches them.
LAST_RESULTS = None


def _trace_kwargs():
    if os.environ.get("BASS_KERNEL_TRACE") == "1":
        cores = os.environ.get("BASS_KERNEL_TRACE_CORES", "0")
        return dict(trace=True, trace_cores=[int(c) for c in cores.split(",")])
    return {}


def kernel(x: np.ndarray, W: np.ndarray, R: np.ndarray) -> np.ndarray:
    global LAST_RESULTS
    x = np.asarray(x, dtype=np.float32)
    W = np.asarray(W, dtype=np.float32)
    R = np.asarray(R, dtype=np.float32)

    # softmax over n (65K elements -- host)
    Rm = R.max(axis=0, keepdims=True)
    e = np.exp(R - Rm)
    Rs = (e / e.sum(axis=0, keepdims=True)).astype(np.float32)

    # upload layouts: x^T as [n, i, B]; W scaled by softmax(R) and laid out
    # i-major as [n, i, k, d]; both in the kernel's bf16 compute precision
    Xp = np.ascontiguousarray(x.transpose(1, 2, 0)).reshape(N, F_X).astype(NPBF16)
    Wsc = W * Rs[:, :, None, None]
    Wp = np.ascontiguousarray(Wsc.transpose(0, 3, 1, 2)).reshape(N, F_W).astype(NPBF16)
    in_maps = []
    for c in range(NCORES):
        sl = slice(c * NL, (c + 1) * NL)
        in_maps.append({"xs": Xp[sl], "ws": Wp[sl]})

    if "nc" not in _CACHE:
        _CACHE["nc"] = build_bass()
    nc = _CACHE["nc"]

    res = run_bass_kernel_spmd(
        nc, in_maps, core_ids=list(range(NCORES)), **_trace_kwargs()
    )
    LAST_RESULTS = res

    s = np.zeros((B, KD), np.float32)
    for r in res.results:
        s += r["out"].astype(np.float32)
    s = s.reshape(B, K, D)
    sq = np.sum(np.square(s), axis=-1, keepdims=True) + EPS
    v = (np.sqrt(sq) / (1.0 + sq)) * s
    return v.astype(np.float32)


if __name__ == "__main__":
    rng = np.random.default_rng(0)
    x = rng.standard_normal((B, N, I), dtype=np.float32)
    W = (rng.standard_normal((N, K, D, I), dtype=np.float32) * 0.05).astype(np.float32)
    R = rng.standard_normal((N, K), dtype=np.float32)
    out = kernel(x, W, R)
    print("out", out.shape, out.dtype, float(np.abs(out).mean()))
